# revision 13
# baseline (speedup 1.0000x reference)
"""Fused BN(inference)+ReLU -> 1x1 conv (512->256) -> 2x2 avgpool on 8 TRN2 cores.

Full inputs in, full output out. Data-parallel over batch (16 -> 2 per core),
BN params + conv weights replicated.

Math folding (host side, tiny):
  s = bn_weight / sqrt(bn_var + eps)            [512]
  t = bn_bias - bn_mean * s                     [512]
  y = relu(s * x + t)                           (one ACT op per channel tile)
  avgpool2x2(W @ y) == (0.25 * W) @ sumpool2x2(y)   (pool before matmul: 4x
                                                     fewer matmul FLOPs)
  wt = 0.25 * W.T                               [512, 256] (lhsT layout)

Engine plan (per core; the HBM stream is the roofline at ~14.3 MB):
  sync  ring: the x stream ONLY (first chunk split in half, last chunk in
              quarters, so the ACT head starts early / the tail drains
              fast). Nothing else queues here -> the stream never stalls.
  scalar ring: params first (st is tiny: unblocks the ACT table load and
              the first RELU; fp16 wt follows), then one RELU per chunk,
              then out DMAs placed late enough in program order that their
              waits are satisfied when the sequencer reaches them.
  DVE/scalar: 2x2 sum-pool in fp16 (2x DVE rate) + PSUM->SBUF copies.
  PE:         fp16 matmuls (single pass + fast weight load; fp32 matmuls
              lower to TWO HI/LO passes and serialize ~33us of PE time),
              fp32 PSUM accumulate.

y/wt are rounded to fp16 (inputs and output stay fp32): worst-case output
rel err ~3e-4 against the fp32 reference, well inside the 2e-2 gate.
"""

import copy as _copy

import numpy as np

import bass_rust
import concourse.bass as bass
import concourse.mybir as mybir
import concourse.tile as tile_mod
from concourse.bass_utils import run_bass_kernel_spmd

EPS = 1e-5

B, C_IN, C_OUT, H, W = 16, 512, 256, 56, 56
N_CORES = 8
B_PC = B // N_CORES          # batches per core
HW = H * W                   # 3136
HWP = (H // 2) * (W // 2)    # 784 pooled spatial
K_TILES = C_IN // 128        # 4
M_TILES = C_OUT // 128       # 2
N_CHUNK = HWP // 2           # 392 (fits one PSUM bank)

_DT = mybir.dt.float32
_DTH = mybir.dt.float16


# This walrus build enforces per-instruction sync-wait caps that Tile's
# add_semaphores pass does not respect: CTRL-type instructions (Drain, NoOp)
# take no sem-ge waits at all, EventSemaphore takes at most 2, and every
# other instruction takes at most 1. Post-pass: hoist excess waits onto
# EventSemaphore carrier instructions inserted just before the owning
# instruction on the same engine (same blocking semantics - the carrier
# blocks the engine's sequencer until its waits pass).
_CTRL_OPS = ("InstDrain", "InstNoOp")


def _hoist_excess_waits(nc):
    ev_counter = [0]

    def make_carrier(engine, waits):
        ev_counter[0] += 1
        return mybir.InstEventSemaphore(
            name=f"EVHOIST-{ev_counter[0]}",
            engine=engine,
            ins=[],
            outs=[],
            sync_info=bass_rust.SyncInfo(on_wait=waits, on_update=[]),
        )

    new_module = _copy.replace(nc.m, functions=[])
    for function in nc.m.functions:
        new_function = _copy.replace(function, blocks=[])
        new_function.set_allocations_from_list(function.allocations)
        for block in function.blocks:
            new_insts = []
            for ins in block.instructions:
                si = ins.sync_info
                waits = list(si.on_wait) if si is not None else []
                opname = type(ins).__name__
                if opname in _CTRL_OPS:
                    keep = [w for w in waits if w.wait_mode != "sem-ge-imm"]
                    excess = [w for w in waits if w.wait_mode == "sem-ge-imm"]
                else:
                    limit = 2 if opname == "InstEventSemaphore" else 1
                    keep, excess = waits[:limit], waits[limit:]
                if excess:
                    for i in range(0, len(excess), 2):
                        new_insts.append(make_carrier(ins.engine, excess[i : i + 2]))
                    si.on_wait = keep
                new_insts.append(ins)
            new_function.blocks.append(_copy.replace(block, instructions=new_insts))
        new_module.functions.append(new_function)
    nc.m = new_module


def build_bass():
    nc = bass.Bass()

    # Params come pre-transposed from the host into partition-major layouts so
    # their DMAs are fully contiguous. st packs s and t ([128, 2K], 4KB) so
    # one tiny DMA unblocks both the ACT table warm-up and the first RELU.
    x_d = nc.dram_tensor("x", [B_PC, C_IN, H, W], _DT, kind="ExternalInput")
    st_d = nc.dram_tensor("st", [128, 2 * K_TILES], _DT, kind="ExternalInput")
    wt_d = nc.dram_tensor(
        "wt", [128, K_TILES * C_OUT], _DTH, kind="ExternalInput"
    )
    out_d = nc.dram_tensor(
        "out", [B_PC, C_OUT, H // 2, W // 2], _DT, kind="ExternalOutput"
    )

    with tile_mod.TileContext(nc) as tc:
        with (
            tc.tile_pool(name="const", bufs=1) as cpool,
            tc.tile_pool(name="xs", bufs=9) as xpool,
            tc.tile_pool(name="ys", bufs=5) as ypool,
            tc.tile_pool(name="us", bufs=4) as upool,
            tc.tile_pool(name="ps", bufs=4) as ppool,
            tc.tile_pool(name="os", bufs=4) as opool,
            tc.tile_pool(name="psum", bufs=8, space="PSUM") as pspool,
        ):
            # Params on the SCALAR HWDGE ring: they must not queue behind the
            # 1.6MB x chunks on sync (that held the first RELU + ACT table
            # load hostage for ~8us). st (4KB) lands in <1us.
            # Trigger the lazy ACT Relu table load (~2.7us) as the scalar
            # engine's FIRST instruction: the warm-up reads a memset tile,
            # not st, so the load isn't gated on any DMA, and the st/wt
            # dispatches below run while the table streams in.
            warm_src = cpool.tile([1, 1], _DT)
            nc.vector.memset(warm_src[:], 0.0)
            warm = cpool.tile([1, 1], _DT)
            nc.scalar.activation(
                warm[:], warm_src[:], mybir.ActivationFunctionType.Relu
            )
            st_sb = cpool.tile([128, 2 * K_TILES], _DT)
            nc.scalar.dma_start(out=st_sb[:], in_=st_d[:])
            wt_sb = cpool.tile([128, K_TILES * C_OUT], _DTH)
            nc.scalar.dma_start(out=wt_sb[:], in_=wt_d[:])

            # x stream: sync ring only. First chunk starts with quarters so
            # the ACT head starts early; the stream ENDS with decreasing
            # piece sizes (b1k2 halves, b1k3 quarters) so ACT drains in
            # lockstep with the final arrivals and the post-stream tail is
            # just one quarter's worth of compute.
            pieces = [
                (0, 0, 0, 14), (0, 0, 14, 14), (0, 0, 28, 28),
                (0, 1, 0, H), (0, 2, 0, H), (0, 3, 0, H),
                (1, 0, 0, H), (1, 1, 0, H),
                (1, 2, 0, 28), (1, 2, 28, 28),
                (1, 3, 0, 14), (1, 3, 14, 14), (1, 3, 28, 14), (1, 3, 42, 14),
            ]
            x_tiles = {}
            for b, k, row0, nrows in pieces:
                x_t = xpool.tile(
                    [128, nrows * W], _DT, tag="x", name=f"x_{b}_{k}_{row0}"
                )
                nc.sync.dma_start(
                    out=x_t[:],
                    in_=x_d[
                        b, k * 128 : (k + 1) * 128, row0 : row0 + nrows
                    ].rearrange("ch h w -> ch (h w)"),
                )
                x_tiles[(b, k, row0)] = x_t

            def wt_ap(k, m):
                off = k * C_OUT + m * 128
                return wt_sb[:, off : off + 128]

            psums_by_batch = [{} for _ in range(B_PC)]
            o_tiles = {}

            def get_o(b, m):
                if (b, m) not in o_tiles:
                    o_tiles[(b, m)] = opool.tile(
                        [128, HWP], _DT, tag="o", name=f"o_{b}_{m}"
                    )
                return o_tiles[(b, m)]

            def emit_compute(b, k, row0, nrows):
                """BN+ReLU (fp16 out) -> 2x2 sum-pool -> matmul into psums."""
                psums = psums_by_batch[b]
                first_k = k == 0
                last_k = k == K_TILES - 1
                hc = nrows * W
                x_t = x_tiles[(b, k, row0)]
                c = row0 // 14
                y_t = ypool.tile([128, hc], _DTH, tag="y", name=f"y_{b}_{k}_{c}")
                nc.scalar.activation(
                    y_t[:],
                    x_t[:],
                    mybir.ActivationFunctionType.Relu,
                    bias=st_sb[:, K_TILES + k : K_TILES + k + 1],
                    scale=st_sb[:, k : k + 1],
                )
                # H-pairs first: operands are contiguous 56-elem runs
                u_t = upool.tile(
                    [128, hc // 2], _DTH, tag="u", name=f"u_{b}_{k}_{c}"
                )
                yv = y_t[:].rearrange("p (h two w) -> p h two w", two=2, w=W)
                nc.vector.tensor_add(u_t[:], yv[:, :, 0, :], yv[:, :, 1, :])
                # then W-pairs
                p_t = ppool.tile(
                    [128, hc // 4], _DTH, tag="p", name=f"p_{b}_{k}_{c}"
                )
                uv = u_t[:].rearrange("p (a two) -> p a two", two=2)
                nc.vector.tensor_add(p_t[:], uv[:, :, 0], uv[:, :, 1])
                # map this chunk's pooled columns onto psum n-chunk pieces.
                # PSUM first-write semantics: the matmul covering an n-chunk's
                # column 0 at k==0 carries start=True; later same-k pieces
                # land as overwrites on the cleared has_written bits.
                pooled0 = (row0 // 2) * (W // 2)  # global pooled col offset
                pooled_w = (nrows // 2) * (W // 2)
                for m in range(M_TILES):
                    off = 0
                    while off < pooled_w:
                        g = pooled0 + off  # global pooled col
                        n = g // N_CHUNK
                        col = g % N_CHUNK
                        width = min(N_CHUNK - col, pooled_w - off)
                        if first_k and (m, n) not in psums:
                            psums[(m, n)] = pspool.tile(
                                [128, N_CHUNK],
                                _DT,
                                tag="psum",
                                name=f"psum_{b}_{m}_{n}",
                            )
                        nc.tensor.matmul(
                            psums[(m, n)][:, col : col + width],
                            wt_ap(k, m),
                            p_t[:, off : off + width],
                            start=(first_k and col == 0),
                            stop=(last_k and col + width == N_CHUNK),
                            skip_group_check=True,
                        )
                        off += width

            def emit_psum_copy(b, n, m, eng, col0=0, width=N_CHUNK):
                # PSUM -> SBUF (DMA can't read PSUM). Both n-halves of a
                # (b, m) land in ONE [128, 784] tile. eng picks DVE or ACT:
                # at the tail both are free and the copies run in parallel;
                # mid-stream only DVE is safe (an ACT copy would stall the
                # RELU stream behind its matmul-chain wait).
                psums = psums_by_batch[b]
                dst = get_o(b, m)[
                    :, n * N_CHUNK + col0 : n * N_CHUNK + col0 + width
                ]
                src = psums[(m, n)][:, col0 : col0 + width]
                if eng is nc.vector:
                    nc.vector.tensor_copy(dst, src)
                else:
                    nc.scalar.copy(dst, src)

            out_v = out_d[:].rearrange("bb o h w -> bb o (h w)")

            def emit_out_dma(b, col0, width, eng):
                for m in range(M_TILES):
                    eng.dma_start(
                        out=out_v[b, m * 128 : (m + 1) * 128, col0 : col0 + width],
                        in_=o_tiles[(b, m)][:, col0 : col0 + width],
                    )

            # Program order: compute b0 fully, drain its psums, then b1.
            # b0's out DMAs go on the scalar ring AFTER two of b1's RELUs so
            # the sequencer reaches them with their waits long satisfied.
            # b1's final k-tile arrives in quarters: psum n0 finishes after
            # q1, n1 cols 0..195 after q2 -> copy + ship those early; only
            # the last 196 columns (100KB x2) remain after the final quarter.
            for b, k, row0, nrows in pieces:
                if b == 1 and k == 2 and row0 == 0:
                    # b0's copies (DVE) completed ~2 chunk-times ago: the
                    # scalar sequencer reaches this with waits satisfied.
                    emit_out_dma(0, 0, HWP, nc.scalar)
                emit_compute(b, k, row0, nrows)
                if b == 0 and k == K_TILES - 1:
                    for m in range(M_TILES):
                        emit_psum_copy(0, 0, m, nc.vector)
                        emit_psum_copy(0, 1, m, nc.vector)
                elif b == 1 and k == K_TILES - 1 and row0 == 28:
                    # q2 done -> psum n0 long complete; DVE has an idle slot
                    # between q2's and q3's pools for exactly one copy.
                    emit_psum_copy(1, 0, 0, nc.vector)
                elif b == 1 and k == K_TILES - 1 and row0 == 42:
                    # Tail endgame, everything after the last RELU. ACT is
                    # free now: it drains the already-ready copies while DVE
                    # finishes q3's pools; the final n1 copies split across
                    # both engines; the two full-width out DMAs dispatch in
                    # parallel on separate HWDGE rings.
                    emit_psum_copy(1, 0, 1, nc.scalar)
                    emit_psum_copy(1, 1, 0, nc.scalar, 0, 196)
                    emit_psum_copy(1, 1, 1, nc.scalar, 0, 196)
                    emit_psum_copy(1, 1, 0, nc.vector, 196, 196)
                    emit_psum_copy(1, 1, 1, nc.scalar, 196, 196)
                    for m, eng in ((0, nc.sync), (1, nc.scalar)):
                        eng.dma_start(
                            out=out_v[1, m * 128 : (m + 1) * 128, :],
                            in_=o_tiles[(1, m)][:],
                        )
            assert sum(nr for _, _, _, nr in pieces) == B_PC * K_TILES * H
    _hoist_excess_waits(nc)
    return nc


_NC_CACHE = None


def _get_nc():
    global _NC_CACHE
    if _NC_CACHE is None:
        _NC_CACHE = build_bass()
    return _NC_CACHE


def _prep_host(bn_weight, bn_bias, bn_mean, bn_var, conv_weight):
    s = (bn_weight / np.sqrt(bn_var + EPS)).astype(np.float32)
    t = (bn_bias - bn_mean * s).astype(np.float32)
    wt = (0.25 * conv_weight.T).astype(np.float32)  # [C_IN, C_OUT]
    # partition-major layouts: [128, 2K] for s|t, [128, K*C_OUT] fp16 for wt
    s2 = s.reshape(K_TILES, 128).T
    t2 = t.reshape(K_TILES, 128).T
    st = np.ascontiguousarray(np.concatenate([s2, t2], axis=1))
    wt2 = np.ascontiguousarray(
        wt.reshape(K_TILES, 128, C_OUT).transpose(1, 0, 2).reshape(128, -1)
    ).astype(np.float16)
    return st, wt2


def _install_ntff_hook():
    # The agent image's antenv lacks axon_hooks; synthesize it from the boot
    # shim's ctypes factory so trace=True captures NTFF profiles.
    import sys
    import types

    try:
        import antenv.axon_hooks  # noqa: F401

        return
    except ImportError:
        pass
    from trn_agent_boot.trn_boot import _ntff_profile_via_ctypes

    hook = _ntff_profile_via_ctypes("/opt/axon/libaxon_pjrt.so")
    mod = types.ModuleType("antenv.axon_hooks")
    store = {"h": hook}
    mod.get_axon_ntff_profile_hook = lambda: store["h"]
    mod.set_axon_ntff_profile_hook = lambda h: store.__setitem__("h", h)
    import antenv

    antenv.axon_hooks = mod
    sys.modules["antenv.axon_hooks"] = mod


def kernel(x, bn_weight, bn_bias, bn_mean, bn_var, conv_weight, _trace=False):
    if _trace:
        _install_ntff_hook()
    x = np.asarray(x, dtype=np.float32)
    st, wt = _prep_host(
        np.asarray(bn_weight, dtype=np.float32),
        np.asarray(bn_bias, dtype=np.float32),
        np.asarray(bn_mean, dtype=np.float32),
        np.asarray(bn_var, dtype=np.float32),
        np.asarray(conv_weight, dtype=np.float32),
    )
    in_maps = [
        {"x": np.ascontiguousarray(x[c * B_PC : (c + 1) * B_PC]), "st": st, "wt": wt}
        for c in range(N_CORES)
    ]
    nc = _get_nc()
    res = run_bass_kernel_spmd(
        nc, in_maps, core_ids=list(range(N_CORES)), trace=_trace
    )
    out = np.concatenate([res.results[c]["out"] for c in range(N_CORES)], axis=0)
    if _trace:
        return out, res
    return out


# revision 14
# speedup vs baseline: 1.1485x; 1.1485x over previous
"""Fused BN(inference)+ReLU -> 1x1 conv (512->256) -> 2x2 avgpool on 8 TRN2 cores.

Full inputs in, full output out. Data-parallel over batch (16 -> 2 per core),
BN params + conv weights replicated.

Math folding (host side, tiny):
  s = bn_weight / sqrt(bn_var + eps)            [512]
  t = bn_bias - bn_mean * s                     [512]
  y = relu(s * x + t)                           (one ACT op per channel tile)
  avgpool2x2(W @ y) == (0.25 * W) @ sumpool2x2(y)   (pool before matmul: 4x
                                                     fewer matmul FLOPs)
  wt = 0.25 * W.T                               [512, 256] (lhsT layout)

Engine plan (per core; the HBM stream is the roofline at ~14.3 MB):
  sync  ring: the x stream ONLY (first chunk split in half, last chunk in
              quarters, so the ACT head starts early / the tail drains
              fast). Nothing else queues here -> the stream never stalls.
  scalar ring: params first (st is tiny: unblocks the ACT table load and
              the first RELU; fp16 wt follows), then one RELU per chunk,
              then out DMAs placed late enough in program order that their
              waits are satisfied when the sequencer reaches them.
  DVE/scalar: 2x2 sum-pool in fp16 (2x DVE rate) + PSUM->SBUF copies.
  PE:         fp16 matmuls (single pass + fast weight load; fp32 matmuls
              lower to TWO HI/LO passes and serialize ~33us of PE time),
              fp32 PSUM accumulate.

y/wt are rounded to fp16 (inputs and output stay fp32): worst-case output
rel err ~3e-4 against the fp32 reference, well inside the 2e-2 gate.
"""

import copy as _copy

import numpy as np

import bass_rust
import concourse.bass as bass
import concourse.mybir as mybir
import concourse.tile as tile_mod
from concourse.bass_utils import run_bass_kernel_spmd

EPS = 1e-5

B, C_IN, C_OUT, H, W = 16, 512, 256, 56, 56
N_CORES = 8
B_PC = B // N_CORES          # batches per core
HW = H * W                   # 3136
HWP = (H // 2) * (W // 2)    # 784 pooled spatial
K_TILES = C_IN // 128        # 4
M_TILES = C_OUT // 128       # 2
N_CHUNK = HWP // 2           # 392 (fits one PSUM bank)

_DT = mybir.dt.float32
_DTH = mybir.dt.float16


# This walrus build enforces per-instruction sync-wait caps that Tile's
# add_semaphores pass does not respect: CTRL-type instructions (Drain, NoOp)
# take no sem-ge waits at all, EventSemaphore takes at most 2, and every
# other instruction takes at most 1. Post-pass: hoist excess waits onto
# EventSemaphore carrier instructions inserted just before the owning
# instruction on the same engine (same blocking semantics - the carrier
# blocks the engine's sequencer until its waits pass).
_CTRL_OPS = ("InstDrain", "InstNoOp")


def _hoist_excess_waits(nc):
    ev_counter = [0]

    def make_carrier(engine, waits):
        ev_counter[0] += 1
        return mybir.InstEventSemaphore(
            name=f"EVHOIST-{ev_counter[0]}",
            engine=engine,
            ins=[],
            outs=[],
            sync_info=bass_rust.SyncInfo(on_wait=waits, on_update=[]),
        )

    new_module = _copy.replace(nc.m, functions=[])
    for function in nc.m.functions:
        new_function = _copy.replace(function, blocks=[])
        new_function.set_allocations_from_list(function.allocations)
        for block in function.blocks:
            new_insts = []
            for ins in block.instructions:
                si = ins.sync_info
                waits = list(si.on_wait) if si is not None else []
                opname = type(ins).__name__
                if opname in _CTRL_OPS:
                    keep = [w for w in waits if w.wait_mode != "sem-ge-imm"]
                    excess = [w for w in waits if w.wait_mode == "sem-ge-imm"]
                else:
                    limit = 2 if opname == "InstEventSemaphore" else 1
                    keep, excess = waits[:limit], waits[limit:]
                if excess:
                    for i in range(0, len(excess), 2):
                        new_insts.append(make_carrier(ins.engine, excess[i : i + 2]))
                    si.on_wait = keep
                new_insts.append(ins)
            new_function.blocks.append(_copy.replace(block, instructions=new_insts))
        new_module.functions.append(new_function)
    nc.m = new_module


def build_bass():
    nc = bass.Bass()

    # Params come pre-transposed from the host into partition-major layouts so
    # their DMAs are fully contiguous. st packs s and t ([128, 2K], 4KB) so
    # one tiny DMA unblocks both the ACT table warm-up and the first RELU.
    x_d = nc.dram_tensor("x", [B_PC, C_IN, H, W], _DT, kind="ExternalInput")
    st_d = nc.dram_tensor("st", [128, 2 * K_TILES], _DT, kind="ExternalInput")
    wt_d = nc.dram_tensor(
        "wt", [128, K_TILES * C_OUT], _DTH, kind="ExternalInput"
    )
    out_d = nc.dram_tensor(
        "out", [B_PC, C_OUT, H // 2, W // 2], _DT, kind="ExternalOutput"
    )

    with tile_mod.TileContext(nc) as tc:
        with (
            tc.tile_pool(name="const", bufs=1) as cpool,
            tc.tile_pool(name="xs", bufs=9) as xpool,
            tc.tile_pool(name="ys", bufs=5) as ypool,
            tc.tile_pool(name="us", bufs=4) as upool,
            tc.tile_pool(name="ps", bufs=4) as ppool,
            tc.tile_pool(name="os", bufs=4) as opool,
            tc.tile_pool(name="psum", bufs=8, space="PSUM") as pspool,
        ):
            # Params on the SCALAR HWDGE ring: they must not queue behind the
            # 1.6MB x chunks on sync (that held the first RELU + ACT table
            # load hostage for ~8us). st (4KB) lands in <1us.
            # Scalar-ring head order matters: st's dispatch first (its DMA
            # flies while the table loads), then the warm-up that triggers
            # the lazy ACT Relu table load (~2.7us, reads a memset tile so
            # it's not gated on st landing), then wt (only needed by the
            # first matmul, and the table load blocks the ACT sequencer
            # until it completes anyway).
            st_sb = cpool.tile([128, 2 * K_TILES], _DT)
            nc.scalar.dma_start(out=st_sb[:], in_=st_d[:])
            warm_src = cpool.tile([1, 1], _DT)
            nc.vector.memset(warm_src[:], 0.0)
            warm = cpool.tile([1, 1], _DT)
            nc.scalar.activation(
                warm[:], warm_src[:], mybir.ActivationFunctionType.Relu
            )
            wt_sb = cpool.tile([128, K_TILES * C_OUT], _DTH)
            nc.scalar.dma_start(out=wt_sb[:], in_=wt_d[:])

            # x stream: sync ring only. First chunk starts with quarters so
            # the ACT head starts early; the stream ENDS with decreasing
            # piece sizes (b1k2 halves, b1k3 quarters) so ACT drains in
            # lockstep with the final arrivals and the post-stream tail is
            # just one quarter's worth of compute.
            pieces = [
                (0, 0, 0, 14), (0, 0, 14, 14), (0, 0, 28, 28),
                (0, 1, 0, H), (0, 2, 0, H), (0, 3, 0, H),
                (1, 0, 0, H), (1, 1, 0, H),
                (1, 2, 0, 28), (1, 2, 28, 28),
                (1, 3, 0, 14), (1, 3, 14, 14), (1, 3, 28, 14), (1, 3, 42, 14),
            ]
            x_tiles = {}
            for b, k, row0, nrows in pieces:
                x_t = xpool.tile(
                    [128, nrows * W], _DT, tag="x", name=f"x_{b}_{k}_{row0}"
                )
                nc.sync.dma_start(
                    out=x_t[:],
                    in_=x_d[
                        b, k * 128 : (k + 1) * 128, row0 : row0 + nrows
                    ].rearrange("ch h w -> ch (h w)"),
                )
                x_tiles[(b, k, row0)] = x_t

            def wt_ap(k, m):
                off = k * C_OUT + m * 128
                return wt_sb[:, off : off + 128]

            psums_by_batch = [{} for _ in range(B_PC)]
            o_tiles = {}

            def get_o(b, m):
                if (b, m) not in o_tiles:
                    o_tiles[(b, m)] = opool.tile(
                        [128, HWP], _DT, tag="o", name=f"o_{b}_{m}"
                    )
                return o_tiles[(b, m)]

            def emit_compute(b, k, row0, nrows):
                """BN+ReLU (fp16 out) -> 2x2 sum-pool -> matmul into psums."""
                psums = psums_by_batch[b]
                first_k = k == 0
                last_k = k == K_TILES - 1
                hc = nrows * W
                x_t = x_tiles[(b, k, row0)]
                c = row0 // 14
                y_t = ypool.tile([128, hc], _DTH, tag="y", name=f"y_{b}_{k}_{c}")
                nc.scalar.activation(
                    y_t[:],
                    x_t[:],
                    mybir.ActivationFunctionType.Relu,
                    bias=st_sb[:, K_TILES + k : K_TILES + k + 1],
                    scale=st_sb[:, k : k + 1],
                )
                # H-pairs first: operands are contiguous 56-elem runs
                u_t = upool.tile(
                    [128, hc // 2], _DTH, tag="u", name=f"u_{b}_{k}_{c}"
                )
                yv = y_t[:].rearrange("p (h two w) -> p h two w", two=2, w=W)
                nc.vector.tensor_add(u_t[:], yv[:, :, 0, :], yv[:, :, 1, :])
                # then W-pairs
                p_t = ppool.tile(
                    [128, hc // 4], _DTH, tag="p", name=f"p_{b}_{k}_{c}"
                )
                uv = u_t[:].rearrange("p (a two) -> p a two", two=2)
                nc.vector.tensor_add(p_t[:], uv[:, :, 0], uv[:, :, 1])
                # map this chunk's pooled columns onto psum n-chunk pieces.
                # PSUM first-write semantics: the matmul covering an n-chunk's
                # column 0 at k==0 carries start=True; later same-k pieces
                # land as overwrites on the cleared has_written bits.
                pooled0 = (row0 // 2) * (W // 2)  # global pooled col offset
                pooled_w = (nrows // 2) * (W // 2)
                for m in range(M_TILES):
                    off = 0
                    while off < pooled_w:
                        g = pooled0 + off  # global pooled col
                        n = g // N_CHUNK
                        col = g % N_CHUNK
                        width = min(N_CHUNK - col, pooled_w - off)
                        if first_k and (m, n) not in psums:
                            psums[(m, n)] = pspool.tile(
                                [128, N_CHUNK],
                                _DT,
                                tag="psum",
                                name=f"psum_{b}_{m}_{n}",
                            )
                        nc.tensor.matmul(
                            psums[(m, n)][:, col : col + width],
                            wt_ap(k, m),
                            p_t[:, off : off + width],
                            start=(first_k and col == 0),
                            stop=(last_k and col + width == N_CHUNK),
                            skip_group_check=True,
                        )
                        off += width

            def emit_psum_copy(b, n, m, eng, col0=0, width=N_CHUNK):
                # PSUM -> SBUF (DMA can't read PSUM). Both n-halves of a
                # (b, m) land in ONE [128, 784] tile. eng picks DVE or ACT:
                # at the tail both are free and the copies run in parallel;
                # mid-stream only DVE is safe (an ACT copy would stall the
                # RELU stream behind its matmul-chain wait).
                psums = psums_by_batch[b]
                dst = get_o(b, m)[
                    :, n * N_CHUNK + col0 : n * N_CHUNK + col0 + width
                ]
                src = psums[(m, n)][:, col0 : col0 + width]
                if eng is nc.vector:
                    nc.vector.tensor_copy(dst, src)
                else:
                    nc.scalar.copy(dst, src)

            out_v = out_d[:].rearrange("bb o h w -> bb o (h w)")

            def emit_out_dma(b, col0, width, eng):
                for m in range(M_TILES):
                    eng.dma_start(
                        out=out_v[b, m * 128 : (m + 1) * 128, col0 : col0 + width],
                        in_=o_tiles[(b, m)][:, col0 : col0 + width],
                    )

            # Program order: compute b0 fully, drain its psums, then b1.
            # b0's out DMAs go on the scalar ring AFTER two of b1's RELUs so
            # the sequencer reaches them with their waits long satisfied.
            # b1's final k-tile arrives in quarters: psum n0 finishes after
            # q1, n1 cols 0..195 after q2 -> copy + ship those early; only
            # the last 196 columns (100KB x2) remain after the final quarter.
            for b, k, row0, nrows in pieces:
                if b == 1 and k == 2 and row0 == 0:
                    # b0's copies (DVE) completed ~2 chunk-times ago: the
                    # scalar sequencer reaches this with waits satisfied.
                    emit_out_dma(0, 0, HWP, nc.scalar)
                emit_compute(b, k, row0, nrows)
                if b == 0 and k == K_TILES - 1:
                    for m in range(M_TILES):
                        emit_psum_copy(0, 0, m, nc.vector)
                        emit_psum_copy(0, 1, m, nc.vector)
                elif b == 1 and k == K_TILES - 1 and row0 == 28:
                    # q2 done -> psum n0 long complete; DVE has an idle slot
                    # between q2's and q3's pools for exactly one copy.
                    emit_psum_copy(1, 0, 0, nc.vector)
                elif b == 1 and k == K_TILES - 1 and row0 == 42:
                    # Tail endgame, everything after the last RELU. ACT is
                    # free now: it drains the already-ready copies while DVE
                    # finishes q3's pools; the final n1 copies split across
                    # both engines; the two full-width out DMAs dispatch in
                    # parallel on separate HWDGE rings.
                    emit_psum_copy(1, 0, 1, nc.scalar)
                    emit_psum_copy(1, 1, 0, nc.scalar, 0, 196)
                    emit_psum_copy(1, 1, 1, nc.scalar, 0, 196)
                    emit_psum_copy(1, 1, 0, nc.vector, 196, 196)
                    emit_psum_copy(1, 1, 1, nc.scalar, 196, 196)
                    for m, eng in ((0, nc.sync), (1, nc.scalar)):
                        eng.dma_start(
                            out=out_v[1, m * 128 : (m + 1) * 128, :],
                            in_=o_tiles[(1, m)][:],
                        )
            assert sum(nr for _, _, _, nr in pieces) == B_PC * K_TILES * H
    _hoist_excess_waits(nc)
    return nc


_NC_CACHE = None


def _get_nc():
    global _NC_CACHE
    if _NC_CACHE is None:
        _NC_CACHE = build_bass()
    return _NC_CACHE


def _prep_host(bn_weight, bn_bias, bn_mean, bn_var, conv_weight):
    s = (bn_weight / np.sqrt(bn_var + EPS)).astype(np.float32)
    t = (bn_bias - bn_mean * s).astype(np.float32)
    wt = (0.25 * conv_weight.T).astype(np.float32)  # [C_IN, C_OUT]
    # partition-major layouts: [128, 2K] for s|t, [128, K*C_OUT] fp16 for wt
    s2 = s.reshape(K_TILES, 128).T
    t2 = t.reshape(K_TILES, 128).T
    st = np.ascontiguousarray(np.concatenate([s2, t2], axis=1))
    wt2 = np.ascontiguousarray(
        wt.reshape(K_TILES, 128, C_OUT).transpose(1, 0, 2).reshape(128, -1)
    ).astype(np.float16)
    return st, wt2


def _install_ntff_hook():
    # The agent image's antenv lacks axon_hooks; synthesize it from the boot
    # shim's ctypes factory so trace=True captures NTFF profiles.
    import sys
    import types

    try:
        import antenv.axon_hooks  # noqa: F401

        return
    except ImportError:
        pass
    from trn_agent_boot.trn_boot import _ntff_profile_via_ctypes

    hook = _ntff_profile_via_ctypes("/opt/axon/libaxon_pjrt.so")
    mod = types.ModuleType("antenv.axon_hooks")
    store = {"h": hook}
    mod.get_axon_ntff_profile_hook = lambda: store["h"]
    mod.set_axon_ntff_profile_hook = lambda h: store.__setitem__("h", h)
    import antenv

    antenv.axon_hooks = mod
    sys.modules["antenv.axon_hooks"] = mod


def kernel(x, bn_weight, bn_bias, bn_mean, bn_var, conv_weight, _trace=False):
    if _trace:
        _install_ntff_hook()
    x = np.asarray(x, dtype=np.float32)
    st, wt = _prep_host(
        np.asarray(bn_weight, dtype=np.float32),
        np.asarray(bn_bias, dtype=np.float32),
        np.asarray(bn_mean, dtype=np.float32),
        np.asarray(bn_var, dtype=np.float32),
        np.asarray(conv_weight, dtype=np.float32),
    )
    in_maps = [
        {"x": np.ascontiguousarray(x[c * B_PC : (c + 1) * B_PC]), "st": st, "wt": wt}
        for c in range(N_CORES)
    ]
    nc = _get_nc()
    res = run_bass_kernel_spmd(
        nc, in_maps, core_ids=list(range(N_CORES)), trace=_trace
    )
    out = np.concatenate([res.results[c]["out"] for c in range(N_CORES)], axis=0)
    if _trace:
        return out, res
    return out
